# revision 15
# baseline (speedup 1.0000x reference)
"""Trainium2 Bass kernel: AnatomicalStructureEncoder (histogram binning).

Strategy:
  - 8 cores; core c handles batch b=c//4, patches [(c%4)*4096, +4096).
  - The batch's full segmentation volume (64 MiB int32) is replicated to
    each core's DRAM; patch windows are fetched with one indirect (gather)
    DMA per 16-voxel x-row (64 rows per patch).
  - Layout: patch p (local) = 2048*h + c with h in {0,1}, c in [0,2048).
    SBUF partition q = 64*h + w holds window w = dz*16+dy of patches (h, .).
  - Histogram: per region r, a bf16 is_equal compare (DVE 4x mode) builds a
    one-hot tile; TensorE with a [128,2] block-ones stationary sums the 64
    window-partitions, accumulating the 16 x-positions into the same PSUM
    columns (16 accumulating matmuls); ScalarE drains PSUM and an
    SBUF->SBUF DMA rearranges to a [2R, NCH] counts layout (partition
    index 20*h + r).
  - Normalization uses the algebraic identity
        a_r = (counts_r + 1e-6*numel) / (sum_r counts_r + 20e-6*numel).
"""

import numpy as np

D, H, W = 64, 512, 512
HW = H * W
DHW = D * H * W
B = 2
NPTOT = 16384          # patches per batch
NCORES = 8
NP = 4096              # patches per core
NCH = NP // 2          # 2048 columns per half
CHUNK = 128            # columns gathered/processed per chunk
NCHUNK = NCH // CHUNK  # 8
R = 20
PW = 16                # x window length
GATHER_COLS = 1        # columns per indirect DMA (128 windows each)
EMBED = 768

_built = {}


def _mk_ap(bass, ap_like, pattern):
    return bass.AP(tensor=ap_like.tensor, offset=ap_like.offset, ap=pattern)


def build(debug=False):
    import contextlib

    import concourse.bass as bass
    import concourse.tile as tile
    from concourse import bacc, mybir

    f32 = mybir.dt.float32
    bf16 = mybir.dt.bfloat16
    i32 = mybir.dt.int32
    Alu = mybir.AluOpType

    nc = bacc.Bacc()
    vol = nc.declare_dram_parameter("vol", [DHW], i32, isOutput=False)
    coordsT = nc.declare_dram_parameter("coordsT", [3, 2, NCH], f32, isOutput=False)
    outp = nc.declare_dram_parameter("out", [2 * R, NCH], f32, isOutput=True)
    if debug:
        countsD = nc.declare_dram_parameter("countsD", [2, R, NCH], f32, isOutput=True)
        dbg_ones = nc.declare_dram_parameter("dbg_ones", [128, 2], mybir.dt.bfloat16, isOutput=True)
        dbg_off = nc.declare_dram_parameter("dbg_off", [128, NCH], i32, isOutput=True)
        dbg_lxe = nc.declare_dram_parameter("dbg_lxe", [128, NCH], mybir.dt.bfloat16, isOutput=True)
        dbg_g = nc.declare_dram_parameter("dbg_g", [128, CHUNK, PW], i32, isOutput=True)
        dbg_dm = nc.declare_dram_parameter("dbg_dm", [128, CHUNK, PW], mybir.dt.bfloat16, isOutput=True)
    else:
        countsD = nc.dram_tensor("countsD", [2, R, NCH], mybir.dt.float32)

    with tile.TileContext(nc) as tc, contextlib.ExitStack() as ctx:
        consts = ctx.enter_context(tc.tile_pool(name="consts", bufs=1))
        keep = ctx.enter_context(tc.tile_pool(name="keep", bufs=1))
        mctx = contextlib.ExitStack()
        mathp = mctx.enter_context(tc.tile_pool(name="math", bufs=1))

        # ---- per-partition constants --------------------------------------
        qI = consts.tile([128, 1], i32)
        nc.gpsimd.iota(qI[:], [[1, 1]], channel_multiplier=1)
        dzI = consts.tile([128, 1], i32)
        nc.vector.tensor_scalar(out=dzI[:], in0=qI[:], scalar1=4, scalar2=3,
                                op0=Alu.logical_shift_right, op1=Alu.bitwise_and)
        dyI = consts.tile([128, 1], i32)
        nc.vector.tensor_scalar(out=dyI[:], in0=qI[:], scalar1=15, scalar2=None,
                                op0=Alu.bitwise_and)
        dzF = consts.tile([128, 1], f32)
        nc.vector.tensor_copy(dzF[:], dzI[:])
        dyF = consts.tile([128, 1], f32)
        nc.vector.tensor_copy(dyF[:], dyI[:])
        t01 = consts.tile([128, 1], f32)
        nc.vector.tensor_scalar(out=t01[:], in0=dyF[:], scalar1=float(W),
                                scalar2=None, op0=Alu.mult)
        deltaF = consts.tile([128, 1], f32)
        nc.vector.scalar_tensor_tensor(out=deltaF[:], in0=dzF[:], scalar=float(HW),
                                       in1=t01[:], op0=Alu.mult, op1=Alu.add)

        iotaKI = consts.tile([128, PW], i32)
        nc.gpsimd.iota(iotaKI[:], [[1, PW]], channel_multiplier=0)
        iotaK = consts.tile([128, PW], bf16)
        nc.vector.tensor_copy(iotaK[:], iotaKI[:])

        ones2 = consts.tile([128, 2], bf16)
        nc.vector.memset(ones2[:], 0.0)
        nc.vector.memset(ones2[0:64, 0:1], 1.0)
        nc.vector.memset(ones2[64:128, 1:2], 1.0)

        # counts partition layout: p = 20*h + r (h-major).
        # lhsT40 [40, 2] f32: [20h+r, m] = (m == h)   (sum bins -> S per half)
        p40 = consts.tile([40, 1], i32)
        nc.gpsimd.iota(p40[:], [[1, 1]], channel_multiplier=1)
        p40f = consts.tile([40, 1], f32)
        nc.vector.tensor_copy(p40f[:], p40[:])
        h1c = consts.tile([40, 1], f32)
        nc.vector.tensor_scalar(out=h1c[:], in0=p40f[:], scalar1=19.5,
                                scalar2=None, op0=Alu.is_gt)
        lhsT40 = consts.tile([40, 2], f32)
        nc.vector.tensor_scalar(out=lhsT40[:, 0:1], in0=h1c[:], scalar1=-1.0,
                                scalar2=1.0, op0=Alu.mult, op1=Alu.add)
        nc.vector.tensor_copy(lhsT40[:, 1:2], h1c[:])

        # lhsT240 [2, 40] f32: [h, 20h'+r] = (h == h')  (broadcast half -> bins)
        i240 = consts.tile([2, 40], i32)
        nc.gpsimd.iota(i240[:], [[1, 40]], channel_multiplier=-20)  # v = j - 20h
        i240f = consts.tile([2, 40], f32)
        nc.vector.tensor_copy(i240f[:], i240[:])
        i240a = consts.tile([2, 40], f32)
        nc.vector.tensor_scalar(out=i240a[:], in0=i240f[:], scalar1=-0.5,
                                scalar2=None, op0=Alu.is_gt)
        i240b2 = consts.tile([2, 40], f32)
        nc.vector.tensor_scalar(out=i240b2[:], in0=i240f[:], scalar1=19.5,
                                scalar2=None, op0=Alu.is_lt)
        lhsT240 = consts.tile([2, 40], f32)
        nc.vector.tensor_tensor(out=lhsT240[:], in0=i240a[:], in1=i240b2[:],
                                op=Alu.mult)

        # ---- load coords (replicated across the 64 window slots) ----------
        cF = []
        for comp in range(3):
            t = mathp.tile([128, NCH], f32, tag=f"c{comp}")
            src = coordsT[comp, :, :]
            nc.gpsimd.dma_start(
                out=t[:],
                in_=_mk_ap(bass, src, [[NCH, 2], [0, 64], [1, NCH]]),
            )
            cF.append(t)
        czF, cyF, cxF = cF

        # ---- coords math ---------------------------------------------------
        def floor_of(src, scale, tagp):
            pix = mathp.tile([128, NCH], f32, tag="mt", bufs=8)
            nc.vector.tensor_scalar(out=pix[:], in0=src[:], scalar1=scale,
                                    scalar2=None, op0=Alu.mult)
            fi = mathp.tile([128, NCH], i32, tag="mt", bufs=8)
            nc.vector.tensor_copy(fi[:], pix[:])
            ff = mathp.tile([128, NCH], f32, tag="mt", bufs=8)
            nc.vector.tensor_copy(ff[:], fi[:])
            gt = mathp.tile([128, NCH], f32, tag="mt", bufs=8)
            nc.vector.tensor_tensor(out=gt[:], in0=ff[:], in1=pix[:], op=Alu.is_gt)
            fl = mathp.tile([128, NCH], f32, tag="mt", bufs=8)
            nc.vector.tensor_tensor(out=fl[:], in0=ff[:], in1=gt[:], op=Alu.subtract)
            return fl

        def axis(src, scale, half, dim, tagp):
            fl = floor_of(src, scale, tagp)
            s0 = mathp.tile([128, NCH], f32, tag="s0" + tagp)
            nc.vector.tensor_scalar(out=s0[:], in0=fl[:], scalar1=-half,
                                    scalar2=0.0, op0=Alu.add, op1=Alu.max)
            se = mathp.tile([128, NCH], f32, tag="mt", bufs=8)
            nc.vector.tensor_scalar(out=se[:], in0=fl[:], scalar1=half,
                                    scalar2=dim, op0=Alu.add, op1=Alu.min)
            L = mathp.tile([128, NCH], f32, tag="L" + tagp)
            nc.vector.tensor_tensor(out=L[:], in0=se[:], in1=s0[:], op=Alu.subtract)
            return s0, L

        z0, Lz = axis(czF, float(D), 2.0, float(D), "z")
        y0, Ly = axis(cyF, float(H), 8.0, float(H), "y")
        x0, Lx = axis(cxF, float(W), 8.0, float(W), "x")

        numelF = mathp.tile([128, NCH], f32)
        t1 = mathp.tile([128, NCH], f32, tag="mt", bufs=8)
        nc.vector.tensor_tensor(out=t1[:], in0=Lz[:], in1=Ly[:], op=Alu.mult)
        nc.vector.tensor_tensor(out=numelF[:], in0=t1[:], in1=Lx[:], op=Alu.mult)

        b1 = mathp.tile([128, NCH], f32, tag="mt", bufs=8)
        nc.vector.scalar_tensor_tensor(out=b1[:], in0=y0[:], scalar=float(W),
                                       in1=x0[:], op0=Alu.mult, op1=Alu.add)
        baseF = mathp.tile([128, NCH], f32, tag="mt", bufs=8)
        nc.vector.scalar_tensor_tensor(out=baseF[:], in0=z0[:], scalar=float(HW),
                                       in1=b1[:], op0=Alu.mult, op1=Alu.add)
        offF = mathp.tile([128, NCH], f32, tag="mt", bufs=8)
        nc.vector.tensor_tensor(out=offF[:], in0=baseF[:],
                                in1=deltaF[:, 0:1].to_broadcast([128, NCH]),
                                op=Alu.add)

        vz = mathp.tile([128, NCH], f32, tag="mt", bufs=8)
        nc.vector.tensor_tensor(out=vz[:], in0=dzF[:, 0:1].to_broadcast([128, NCH]),
                                in1=Lz[:], op=Alu.is_ge)
        vy = mathp.tile([128, NCH], f32, tag="mt", bufs=8)
        nc.vector.tensor_tensor(out=vy[:], in0=dyF[:, 0:1].to_broadcast([128, NCH]),
                                in1=Ly[:], op=Alu.is_ge)
        vinv = mathp.tile([128, NCH], f32, tag="mt", bufs=8)
        nc.vector.tensor_tensor(out=vinv[:], in0=vz[:], in1=vy[:], op=Alu.max)

        offF2 = mathp.tile([128, NCH], f32, tag="mt", bufs=8)
        nc.vector.scalar_tensor_tensor(out=offF2[:], in0=vinv[:], scalar=33554432.0,
                                       in1=offF[:], op0=Alu.mult, op1=Alu.add)
        offI = keep.tile([128, NCH], i32)
        nc.vector.tensor_copy(offI[:], offF2[:])

        vval = mathp.tile([128, NCH], f32, tag="mt", bufs=8)
        nc.vector.tensor_scalar(out=vval[:], in0=vinv[:], scalar1=-1.0,
                                scalar2=1.0, op0=Alu.mult, op1=Alu.add)
        lxe = mathp.tile([128, NCH], f32, tag="mt", bufs=8)
        nc.vector.tensor_tensor(out=lxe[:], in0=Lx[:], in1=vval[:], op=Alu.mult)
        LxE = keep.tile([128, NCH], bf16)
        nc.vector.tensor_copy(LxE[:], lxe[:])

        # numel per (h, c) compacted to 2 partitions
        NUM2 = keep.tile([2, NCH], f32)
        nc.sync.dma_start(out=NUM2[0:1, :], in_=numelF[0:1, :])
        nc.sync.dma_start(out=NUM2[1:2, :], in_=numelF[64:65, :])

        # ---- gather + histogram chunks ------------------------------------
        mctx.close()  # release coords-math SBUF
        gpool = ctx.enter_context(tc.tile_pool(name="g", bufs=3))
        dpool = ctx.enter_context(tc.tile_pool(name="d", bufs=2))
        ohpool = ctx.enter_context(tc.tile_pool(name="oh", bufs=4))
        fpool = ctx.enter_context(tc.tile_pool(name="fold", bufs=2))
        pspool = ctx.enter_context(tc.tile_pool(name="ps", bufs=1, space="PSUM"))
        for j in range(NCHUNK):
            sl = slice(j * CHUNK, (j + 1) * CHUNK)
            G = gpool.tile([128, CHUNK, PW], i32, tag="G")
            # HW-validated indirect gather shape: 128 indices (one per
            # partition) per instruction -> sub-gather width 1 column.
            csub = GATHER_COLS
            nsub = CHUNK // csub
            gbase = G[:]
            for s in range(nsub):
                ssub = slice(j * CHUNK + s * csub, j * CHUNK + (s + 1) * csub)
                # 2-D out AP (the only shape walrus lowers correctly):
                # partition-dim + 16 contiguous elems, offset to column s.
                out2d = _mk_ap(bass, gbase, [gbase.ap[0], [1, PW]])
                out2d = bass.AP(tensor=out2d.tensor,
                                offset=gbase.offset + s * csub * PW,
                                ap=[gbase.ap[0], [1, PW]])
                nc.gpsimd.indirect_dma_start(
                    out=out2d,
                    out_offset=None,
                    in_=_mk_ap(bass, vol[:], [[1, DHW], [1, 1]]),
                    in_offset=bass.IndirectOffsetOnAxis(ap=offI[:, ssub], axis=0),
                    bounds_check=DHW - 1,
                    oob_is_err=False,
                )
            dbf = dpool.tile([128, CHUNK, PW], bf16, tag="dbf")
            nc.vector.tensor_copy(dbf[:], G[:])
            m = dpool.tile([128, CHUNK, PW], bf16, tag="m")
            lxe_b = _mk_ap(bass, LxE[:, sl],
                           [LxE[:, sl].ap[0], [1, CHUNK], [0, PW]])
            iota_b = _mk_ap(bass, iotaK[:, :],
                            [iotaK[:, :].ap[0], [0, CHUNK], [1, PW]])
            nc.vector.tensor_tensor(out=m[:], in0=lxe_b, in1=iota_b, op=Alu.is_gt)
            dm = dpool.tile([128, CHUNK, PW], bf16, tag="dm")
            nc.vector.tensor_tensor(out=dm[:], in0=dbf[:], in1=m[:], op=Alu.mult)
            if debug and j == 0:
                nc.sync.dma_start(out=dbg_g[:], in_=G[:])
                nc.sync.dma_start(out=dbg_dm[:], in_=dm[:])

            # ps[h, r, c] accumulates sum over 64 window-partitions (matmul
            # K-reduce) and 16 x-positions (PSUM accumulation).
            ps = pspool.tile([2, R, CHUNK], f32, tag="ps")
            for r in range(1, R + 1):
                oh = ohpool.tile([128, CHUNK, PW], bf16, tag="oh")
                nc.vector.tensor_scalar(out=oh[:], in0=dm[:], scalar1=float(r),
                                        scalar2=None, op0=Alu.is_equal)
                for k in range(PW):
                    nc.tensor.matmul(
                        out=ps[:, r - 1, :],
                        lhsT=ones2[:],
                        rhs=oh[:, :, k:k + 1],
                        start=(k == 0), stop=(k == PW - 1),
                    )
            # drain PSUM -> SBUF (ScalarE), then rearrange (h, r, c) ->
            # counts[20h + r, c] with an SBUF->SBUF DMA.
            stage = fpool.tile([2, R, CHUNK], f32, tag="stage")
            nc.scalar.activation(stage[:], ps[:],
                                 mybir.ActivationFunctionType.Copy)
            nc.sync.dma_start(out=countsD[:, :, sl], in_=stage[:])

        # ---- normalization -------------------------------------------------
        counts = keep.tile([2 * R, NCH], f32)
        for s4 in range(4):
            ssl = slice(s4 * 512, (s4 + 1) * 512)
            nc.sync.dma_start(
                out=counts[:, ssl],
                in_=_mk_ap(bass, countsD[:, :, ssl],
                           [[R * NCH, 2], [NCH, R], [1, 512]]),
            )
        psS = pspool.tile([2, NCH], f32, tag="ps")
        for s4 in range(4):
            ssl = slice(s4 * 512, (s4 + 1) * 512)
            nc.tensor.matmul(out=psS[:, ssl], lhsT=lhsT40[:], rhs=counts[:, ssl],
                             start=True, stop=True)
        den2 = keep.tile([2, NCH], f32)
        nc.vector.scalar_tensor_tensor(out=den2[:], in0=NUM2[:], scalar=2.0e-5,
                                       in1=psS[:], op0=Alu.mult, op1=Alu.add)
        rcp2 = keep.tile([2, NCH], f32)
        nc.vector.reciprocal(rcp2[:], den2[:])

        psN = pspool.tile([2 * R, NCH], f32, tag="ps")
        for s4 in range(4):
            ssl = slice(s4 * 512, (s4 + 1) * 512)
            nc.tensor.matmul(out=psN[:, ssl], lhsT=lhsT240[:], rhs=NUM2[:, ssl],
                             start=True, stop=True)
        numer = keep.tile([2 * R, NCH], f32)
        for s4 in range(4):
            ssl = slice(s4 * 512, (s4 + 1) * 512)
            nc.vector.scalar_tensor_tensor(out=numer[:, ssl], in0=psN[:, ssl],
                                           scalar=1.0e-6, in1=counts[:, ssl],
                                           op0=Alu.mult, op1=Alu.add)

        psR = pspool.tile([2 * R, NCH], f32, tag="ps")
        for s4 in range(4):
            ssl = slice(s4 * 512, (s4 + 1) * 512)
            nc.tensor.matmul(out=psR[:, ssl], lhsT=lhsT240[:], rhs=rcp2[:, ssl],
                             start=True, stop=True)
        a40 = keep.tile([2 * R, NCH], f32)
        for s4 in range(4):
            ssl = slice(s4 * 512, (s4 + 1) * 512)
            nc.vector.tensor_tensor(out=a40[:, ssl], in0=numer[:, ssl],
                                    in1=psR[:, ssl], op=Alu.mult)
        nc.sync.dma_start(out=outp[:], in_=a40[:])
        if debug:
            nc.sync.dma_start(out=dbg_ones[:], in_=ones2[:])
            nc.sync.dma_start(out=dbg_off[:], in_=offI[:])
            nc.sync.dma_start(out=dbg_lxe[:], in_=LxE[:])

    nc.finalize()
    return nc


def make_in_maps(segmentation_mask, patch_coords):
    mask = np.asarray(segmentation_mask)
    coords = np.asarray(patch_coords)
    in_maps = []
    for core in range(NCORES):
        b = core // 4
        p0 = (core % 4) * NP
        volv = np.ascontiguousarray(mask[b, 0].reshape(-1)).astype(np.int32)
        csh = coords[b, p0:p0 + NP, :].astype(np.float32)        # [NP, 3]
        ct = np.ascontiguousarray(csh.T).reshape(3, 2, NCH).copy()
        in_maps.append({"vol": volv, "coordsT": ct})
    return in_maps


def assemble(results, region_prototypes):
    protos = np.asarray(region_prototypes).astype(np.float32)
    p2r = np.zeros((B, NPTOT, R), np.float32)
    for core in range(NCORES):
        o = np.asarray(results[core]["out"])                     # [2R, NCH]
        bidx = core // 4
        p0 = (core % 4) * NP
        arr = o.reshape(2, R, NCH).transpose(0, 2, 1).reshape(NP, R)
        p2r[bidx, p0:p0 + NP, :] = arr
    region_features = np.broadcast_to(protos[None], (B, R, EMBED)).copy()
    return region_features, p2r


def kernel(segmentation_mask, patch_coords, region_prototypes):
    import sys
    if "/opt/trn_rl_repo" not in sys.path:
        sys.path.insert(0, "/opt/trn_rl_repo")
    from concourse.bass_utils import run_bass_kernel_spmd

    if "nc" not in _built:
        _built["nc"] = build()
    nc = _built["nc"]
    in_maps = make_in_maps(segmentation_mask, patch_coords)
    res = run_bass_kernel_spmd(nc, in_maps, list(range(NCORES))).results
    return assemble(res, region_prototypes)


# revision 17
# speedup vs baseline: 1.3472x; 1.3472x over previous
"""Trainium2 Bass kernel: AnatomicalStructureEncoder (histogram binning).

Pair-block gather variant:
  - 8 cores; core c handles batch b=c//4, patches [(c%4)*4096, +4096).
  - Patch p (local) = 2048*h + c', c' = 1024*gamma + c2.
  - SBUF partition q = 64h + 32*gamma + w2, w2 = dz*8 + ty: holds the
    528-voxel pair-block (rows dy=2ty and 2ty+1 of slice z0+dz) of patch
    (h, gamma, c2-column). One indirect DMA per c2 column moves 128
    blocks (one per partition) -> 1024 gather instructions total.
  - Window j (j=0,1) of a block sits at block offset 512*j..+16; a strided
    cast-copy compacts blocks to dm[128, c2, 2, 16] bf16, masks zero the
    x-tail and invalid windows, per-region is_equal one-hots are reduced
    by TensorE (block-ones stationary, 32 accumulating matmuls fold j,x).
  - Normalization: a_r = (counts_r + 1e-6*numel)/(sum counts + 2e-5*numel).
"""

import numpy as np

D, H, W = 64, 512, 512
HW = H * W
DHW = D * H * W
VPAD = 1024            # DRAM tail pad so 528-elem blocks never overrun
B = 2
NPTOT = 16384          # patches per batch
NCORES = 8
NP = 4096              # patches per core
NCH = NP // 2          # 2048 c' columns per half
C2 = NCH // 2          # 1024 c2 columns
CC = 128               # c2 columns per compute chunk
NCHUNK = C2 // CC      # 8
CG = 8                 # c2 columns per gather tile
BLK = 528              # pair-block elements (512 + 16)
R = 20
PW = 16
EMBED = 768

_built = {}


def _mk_ap(bass, ap_like, pattern, extra_off=0):
    return bass.AP(tensor=ap_like.tensor, offset=ap_like.offset + extra_off,
                   ap=pattern)


def build(debug=False):
    import contextlib

    import concourse.bass as bass
    import concourse.tile as tile
    from concourse import bacc, mybir

    f32 = mybir.dt.float32
    bf16 = mybir.dt.bfloat16
    i32 = mybir.dt.int32
    Alu = mybir.AluOpType

    nc = bacc.Bacc()
    vol = nc.declare_dram_parameter("vol", [DHW + VPAD], i32, isOutput=False)
    coordsT = nc.declare_dram_parameter("coordsT", [3, 2, NCH], f32, isOutput=False)
    outp = nc.declare_dram_parameter("out", [2 * R, NCH], f32, isOutput=True)
    countsD = nc.dram_tensor("countsD", [2, 2, R, C2], mybir.dt.float32)

    with tile.TileContext(nc) as tc, contextlib.ExitStack() as ctx:
        consts = ctx.enter_context(tc.tile_pool(name="consts", bufs=1))
        keep = ctx.enter_context(tc.tile_pool(name="keep", bufs=1))
        mctx = contextlib.ExitStack()
        mathp = mctx.enter_context(tc.tile_pool(name="math", bufs=1))

        # ---- per-partition constants --------------------------------------
        # q = 64h + 32g + 8dz + ty
        qI = consts.tile([128, 1], i32)
        nc.gpsimd.iota(qI[:], [[1, 1]], channel_multiplier=1)
        dzI = consts.tile([128, 1], i32)
        nc.vector.tensor_scalar(out=dzI[:], in0=qI[:], scalar1=3, scalar2=3,
                                op0=Alu.logical_shift_right, op1=Alu.bitwise_and)
        dy0I = consts.tile([128, 1], i32)
        nc.vector.tensor_scalar(out=dy0I[:], in0=qI[:], scalar1=7, scalar2=1,
                                op0=Alu.bitwise_and, op1=Alu.logical_shift_left)
        dzF = consts.tile([128, 1], f32)
        nc.vector.tensor_copy(dzF[:], dzI[:])
        dy0F = consts.tile([128, 1], f32)
        nc.vector.tensor_copy(dy0F[:], dy0I[:])
        dy1F = consts.tile([128, 1], f32)
        nc.vector.tensor_scalar(out=dy1F[:], in0=dy0F[:], scalar1=1.0,
                                scalar2=None, op0=Alu.add)
        t01 = consts.tile([128, 1], f32)
        nc.vector.tensor_scalar(out=t01[:], in0=dy0F[:], scalar1=float(W),
                                scalar2=None, op0=Alu.mult)
        deltaF = consts.tile([128, 1], f32)
        nc.vector.scalar_tensor_tensor(out=deltaF[:], in0=dzF[:], scalar=float(HW),
                                       in1=t01[:], op0=Alu.mult, op1=Alu.add)

        iotaKI = consts.tile([128, PW], i32)
        nc.gpsimd.iota(iotaKI[:], [[1, PW]], channel_multiplier=0)
        iotaK = consts.tile([128, PW], bf16)
        nc.vector.tensor_copy(iotaK[:], iotaKI[:])

        # ones4 [128, 4] bf16: column m = (q >> 5) = 2h + gamma
        hgI = consts.tile([128, 1], i32)
        nc.vector.tensor_scalar(out=hgI[:], in0=qI[:], scalar1=5, scalar2=None,
                                op0=Alu.logical_shift_right)
        hgF = consts.tile([128, 1], f32)
        nc.vector.tensor_copy(hgF[:], hgI[:])
        ones4 = consts.tile([128, 4], bf16)
        for m in range(4):
            nc.vector.tensor_scalar(out=ones4[:, m:m + 1], in0=hgF[:],
                                    scalar1=float(m), scalar2=None,
                                    op0=Alu.is_equal)

        # counts partition layout: 20h + r.
        p40 = consts.tile([40, 1], i32)
        nc.gpsimd.iota(p40[:], [[1, 1]], channel_multiplier=1)
        p40f = consts.tile([40, 1], f32)
        nc.vector.tensor_copy(p40f[:], p40[:])
        h1c = consts.tile([40, 1], f32)
        nc.vector.tensor_scalar(out=h1c[:], in0=p40f[:], scalar1=19.5,
                                scalar2=None, op0=Alu.is_gt)
        lhsT40 = consts.tile([40, 2], f32)
        nc.vector.tensor_scalar(out=lhsT40[:, 0:1], in0=h1c[:], scalar1=-1.0,
                                scalar2=1.0, op0=Alu.mult, op1=Alu.add)
        nc.vector.tensor_copy(lhsT40[:, 1:2], h1c[:])

        i240 = consts.tile([2, 40], i32)
        nc.gpsimd.iota(i240[:], [[1, 40]], channel_multiplier=-20)
        i240f = consts.tile([2, 40], f32)
        nc.vector.tensor_copy(i240f[:], i240[:])
        i240a = consts.tile([2, 40], f32)
        nc.vector.tensor_scalar(out=i240a[:], in0=i240f[:], scalar1=-0.5,
                                scalar2=None, op0=Alu.is_gt)
        i240b2 = consts.tile([2, 40], f32)
        nc.vector.tensor_scalar(out=i240b2[:], in0=i240f[:], scalar1=19.5,
                                scalar2=None, op0=Alu.is_lt)
        lhsT240 = consts.tile([2, 40], f32)
        nc.vector.tensor_tensor(out=lhsT240[:], in0=i240a[:], in1=i240b2[:],
                                op=Alu.mult)

        # ---- coords, replicated: partition group (h, g) reads its c2 slice -
        cF = []
        for comp in range(3):
            t = mathp.tile([128, C2], f32, tag=f"c{comp}")
            for h in (0, 1):
                src = coordsT[comp, h, :]
                nc.gpsimd.dma_start(
                    out=t[64 * h:64 * h + 64, :],
                    in_=_mk_ap(bass, src, [[C2, 2], [0, 32], [1, C2]]),
                )
            cF.append(t)
        czF, cyF, cxF = cF

        # ---- coords math ---------------------------------------------------
        def floor_of(src, scale):
            pix = mathp.tile([128, C2], f32, tag="mt", bufs=8)
            nc.vector.tensor_scalar(out=pix[:], in0=src[:], scalar1=scale,
                                    scalar2=None, op0=Alu.mult)
            fi = mathp.tile([128, C2], i32, tag="mt", bufs=8)
            nc.vector.tensor_copy(fi[:], pix[:])
            ff = mathp.tile([128, C2], f32, tag="mt", bufs=8)
            nc.vector.tensor_copy(ff[:], fi[:])
            gt = mathp.tile([128, C2], f32, tag="mt", bufs=8)
            nc.vector.tensor_tensor(out=gt[:], in0=ff[:], in1=pix[:], op=Alu.is_gt)
            fl = mathp.tile([128, C2], f32, tag="mt", bufs=8)
            nc.vector.tensor_tensor(out=fl[:], in0=ff[:], in1=gt[:], op=Alu.subtract)
            return fl

        def axis(src, scale, half, dim, tagp):
            fl = floor_of(src, scale)
            s0 = mathp.tile([128, C2], f32, tag="s0" + tagp)
            nc.vector.tensor_scalar(out=s0[:], in0=fl[:], scalar1=-half,
                                    scalar2=0.0, op0=Alu.add, op1=Alu.max)
            se = mathp.tile([128, C2], f32, tag="mt", bufs=8)
            nc.vector.tensor_scalar(out=se[:], in0=fl[:], scalar1=half,
                                    scalar2=dim, op0=Alu.add, op1=Alu.min)
            L = mathp.tile([128, C2], f32, tag="L" + tagp)
            nc.vector.tensor_tensor(out=L[:], in0=se[:], in1=s0[:], op=Alu.subtract)
            return s0, L

        z0, Lz = axis(czF, float(D), 2.0, float(D), "z")
        y0, Ly = axis(cyF, float(H), 8.0, float(H), "y")
        x0, Lx = axis(cxF, float(W), 8.0, float(W), "x")

        numelF = mathp.tile([128, C2], f32)
        t1 = mathp.tile([128, C2], f32, tag="mt", bufs=8)
        nc.vector.tensor_tensor(out=t1[:], in0=Lz[:], in1=Ly[:], op=Alu.mult)
        nc.vector.tensor_tensor(out=numelF[:], in0=t1[:], in1=Lx[:], op=Alu.mult)

        b1 = mathp.tile([128, C2], f32, tag="mt", bufs=8)
        nc.vector.scalar_tensor_tensor(out=b1[:], in0=y0[:], scalar=float(W),
                                       in1=x0[:], op0=Alu.mult, op1=Alu.add)
        baseF = mathp.tile([128, C2], f32, tag="mt", bufs=8)
        nc.vector.scalar_tensor_tensor(out=baseF[:], in0=z0[:], scalar=float(HW),
                                       in1=b1[:], op0=Alu.mult, op1=Alu.add)
        offF = mathp.tile([128, C2], f32, tag="mt", bufs=8)
        nc.vector.tensor_tensor(out=offF[:], in0=baseF[:],
                                in1=deltaF[:, 0:1].to_broadcast([128, C2]),
                                op=Alu.add)

        vz = mathp.tile([128, C2], f32, tag="vz")
        nc.vector.tensor_tensor(out=vz[:], in0=dzF[:, 0:1].to_broadcast([128, C2]),
                                in1=Lz[:], op=Alu.is_ge)
        vy0 = mathp.tile([128, C2], f32, tag="vy0")
        nc.vector.tensor_tensor(out=vy0[:], in0=dy0F[:, 0:1].to_broadcast([128, C2]),
                                in1=Ly[:], op=Alu.is_ge)
        vy1 = mathp.tile([128, C2], f32, tag="vy1")
        nc.vector.tensor_tensor(out=vy1[:], in0=dy1F[:, 0:1].to_broadcast([128, C2]),
                                in1=Ly[:], op=Alu.is_ge)
        vinv0 = mathp.tile([128, C2], f32, tag="vinv0")
        nc.vector.tensor_tensor(out=vinv0[:], in0=vz[:], in1=vy0[:], op=Alu.max)
        vinv1 = mathp.tile([128, C2], f32, tag="vinv1")
        nc.vector.tensor_tensor(out=vinv1[:], in0=vz[:], in1=vy1[:], op=Alu.max)

        offF2 = mathp.tile([128, C2], f32, tag="mt", bufs=8)
        nc.vector.scalar_tensor_tensor(out=offF2[:], in0=vinv0[:], scalar=33554432.0,
                                       in1=offF[:], op0=Alu.mult, op1=Alu.add)
        offI = keep.tile([128, C2], i32)
        nc.vector.tensor_copy(offI[:], offF2[:])

        def lxe_of(vinv, tagn):
            vval = mathp.tile([128, C2], f32, tag="mt", bufs=8)
            nc.vector.tensor_scalar(out=vval[:], in0=vinv[:], scalar1=-1.0,
                                    scalar2=1.0, op0=Alu.mult, op1=Alu.add)
            lxe = mathp.tile([128, C2], f32, tag="mt", bufs=8)
            nc.vector.tensor_tensor(out=lxe[:], in0=Lx[:], in1=vval[:], op=Alu.mult)
            LxE = keep.tile([128, C2], bf16, tag=tagn)
            nc.vector.tensor_copy(LxE[:], lxe[:])
            return LxE

        LxE0 = lxe_of(vinv0, "lxe0")
        LxE1 = lxe_of(vinv1, "lxe1")

        # numel per (h, c') to 2 partitions: rows {0,32,64,96} -> c' halves
        NUM2 = keep.tile([2, NCH], f32)
        nc.sync.dma_start(out=NUM2[0:1, 0:C2], in_=numelF[0:1, :])
        nc.sync.dma_start(out=NUM2[0:1, C2:NCH], in_=numelF[32:33, :])
        nc.sync.dma_start(out=NUM2[1:2, 0:C2], in_=numelF[64:65, :])
        nc.sync.dma_start(out=NUM2[1:2, C2:NCH], in_=numelF[96:97, :])

        # ---- gather + histogram chunks ------------------------------------
        mctx.close()
        gpool = ctx.enter_context(tc.tile_pool(name="g", bufs=3))
        dpool = ctx.enter_context(tc.tile_pool(name="d", bufs=2))
        ohpool = ctx.enter_context(tc.tile_pool(name="oh", bufs=2))
        fpool = ctx.enter_context(tc.tile_pool(name="fold", bufs=2))
        pspool = ctx.enter_context(tc.tile_pool(name="ps", bufs=1, space="PSUM"))

        for j in range(NCHUNK):          # 8 chunks of CC=128 c2 columns
            c0 = j * CC
            dm = dpool.tile([128, CC, 2, PW], bf16, tag="dm")
            for s in range(CC // CG):    # 16 gather tiles of CG=8 columns
                gc0 = c0 + s * CG
                G2 = gpool.tile([128, CG, BLK], i32, tag="G2")
                g2b = G2[:]
                for u in range(CG):
                    out2d = bass.AP(tensor=g2b.tensor,
                                    offset=g2b.offset + u * BLK,
                                    ap=[g2b.ap[0], [1, BLK]])
                    nc.gpsimd.indirect_dma_start(
                        out=out2d,
                        out_offset=None,
                        in_=_mk_ap(bass, vol[:], [[1, DHW + VPAD], [1, 1]]),
                        in_offset=bass.IndirectOffsetOnAxis(
                            ap=offI[:, gc0 + u:gc0 + u + 1], axis=0),
                        bounds_check=DHW - 1,
                        oob_is_err=False,
                    )
                # compact-cast: dm[:, s*CG + v, j2, k] <- G2[:, v, 512*j2 + k]
                nc.vector.tensor_copy(
                    dm[:, s * CG:(s + 1) * CG, :, :],
                    _mk_ap(bass, G2[:], [g2b.ap[0], [BLK, CG], [512, 2], [1, PW]]),
                )
            m = dpool.tile([128, CC, 2, PW], bf16, tag="m", bufs=1)
            for j2, LxE in ((0, LxE0), (1, LxE1)):
                lxe_b = _mk_ap(bass, LxE[:, c0:c0 + CC],
                               [LxE[:, c0:c0 + CC].ap[0], [1, CC], [0, PW]])
                iota_b = _mk_ap(bass, iotaK[:, :],
                                [iotaK[:, :].ap[0], [0, CC], [1, PW]])
                nc.vector.tensor_tensor(out=m[:, :, j2, :], in0=lxe_b,
                                        in1=iota_b, op=Alu.is_gt)
            dm2 = dpool.tile([128, CC, 2, PW], bf16, tag="dm2")
            nc.vector.tensor_tensor(out=dm2[:], in0=dm[:], in1=m[:], op=Alu.mult)

            ps = pspool.tile([4, R, CC], f32, tag="ps")
            for r in range(1, R + 1):
                oh = ohpool.tile([128, CC, 2, PW], bf16, tag="oh")
                nc.vector.tensor_scalar(out=oh[:], in0=dm2[:], scalar1=float(r),
                                        scalar2=None, op0=Alu.is_equal)
                for j2 in range(2):
                    for k in range(PW):
                        nc.tensor.matmul(
                            out=ps[:, r - 1, :],
                            lhsT=ones4[:],
                            rhs=oh[:, :, j2:j2 + 1, k:k + 1],
                            start=(j2 == 0 and k == 0),
                            stop=(j2 == 1 and k == PW - 1),
                        )
            stage = fpool.tile([4, R, CC], f32, tag="stage")
            nc.scalar.activation(stage[:], ps[:],
                                 mybir.ActivationFunctionType.Copy)
            # stage[m=2h+g, r, c] -> countsD[h, g, r, c0+c]
            nc.sync.dma_start(
                out=_mk_ap(bass, countsD[:, :, :, :],
                           [[2 * R * C2, 2], [R * C2, 2], [C2, R], [1, CC]],
                           extra_off=c0),
                in_=stage[:],
            )

        # ---- normalization -------------------------------------------------
        counts = keep.tile([2 * R, NCH], f32)
        for g in range(2):
            for quarter in range(4):
                q0 = quarter * 256
                nc.sync.dma_start(
                    out=counts[:, g * C2 + q0:g * C2 + q0 + 256],
                    in_=_mk_ap(bass, countsD[:, :, :, :],
                               [[2 * R * C2, 2], [C2, R], [1, 256]],
                               extra_off=g * R * C2 + q0),
                )
        psS = pspool.tile([2, NCH], f32, tag="ps")
        for s4 in range(4):
            ssl = slice(s4 * 512, (s4 + 1) * 512)
            nc.tensor.matmul(out=psS[:, ssl], lhsT=lhsT40[:], rhs=counts[:, ssl],
                             start=True, stop=True)
        den2 = keep.tile([2, NCH], f32)
        nc.vector.scalar_tensor_tensor(out=den2[:], in0=NUM2[:], scalar=2.0e-5,
                                       in1=psS[:], op0=Alu.mult, op1=Alu.add)
        rcp2 = keep.tile([2, NCH], f32)
        nc.vector.reciprocal(rcp2[:], den2[:])

        psN = pspool.tile([2 * R, NCH], f32, tag="ps")
        for s4 in range(4):
            ssl = slice(s4 * 512, (s4 + 1) * 512)
            nc.tensor.matmul(out=psN[:, ssl], lhsT=lhsT240[:], rhs=NUM2[:, ssl],
                             start=True, stop=True)
        numer = keep.tile([2 * R, NCH], f32)
        for s4 in range(4):
            ssl = slice(s4 * 512, (s4 + 1) * 512)
            nc.vector.scalar_tensor_tensor(out=numer[:, ssl], in0=psN[:, ssl],
                                           scalar=1.0e-6, in1=counts[:, ssl],
                                           op0=Alu.mult, op1=Alu.add)

        psR = pspool.tile([2 * R, NCH], f32, tag="ps")
        for s4 in range(4):
            ssl = slice(s4 * 512, (s4 + 1) * 512)
            nc.tensor.matmul(out=psR[:, ssl], lhsT=lhsT240[:], rhs=rcp2[:, ssl],
                             start=True, stop=True)
        a40 = keep.tile([2 * R, NCH], f32)
        for s4 in range(4):
            ssl = slice(s4 * 512, (s4 + 1) * 512)
            nc.vector.tensor_tensor(out=a40[:, ssl], in0=numer[:, ssl],
                                    in1=psR[:, ssl], op=Alu.mult)
        nc.sync.dma_start(out=outp[:], in_=a40[:])

    nc.finalize()
    return nc


def make_in_maps(segmentation_mask, patch_coords):
    mask = np.asarray(segmentation_mask)
    coords = np.asarray(patch_coords)
    in_maps = []
    for core in range(NCORES):
        b = core // 4
        p0 = (core % 4) * NP
        volv = np.ascontiguousarray(mask[b, 0].reshape(-1)).astype(np.int32)
        volv = np.concatenate([volv, np.zeros(VPAD, np.int32)])
        csh = coords[b, p0:p0 + NP, :].astype(np.float32)        # [NP, 3]
        ct = np.ascontiguousarray(csh.T).reshape(3, 2, NCH).copy()
        in_maps.append({"vol": volv, "coordsT": ct})
    return in_maps


def assemble(results, region_prototypes):
    protos = np.asarray(region_prototypes).astype(np.float32)
    p2r = np.zeros((B, NPTOT, R), np.float32)
    for core in range(NCORES):
        o = np.asarray(results[core]["out"])                     # [2R, NCH]
        bidx = core // 4
        p0 = (core % 4) * NP
        arr = o.reshape(2, R, NCH).transpose(0, 2, 1).reshape(NP, R)
        p2r[bidx, p0:p0 + NP, :] = arr
    region_features = np.broadcast_to(protos[None], (B, R, EMBED)).copy()
    return region_features, p2r


def kernel(segmentation_mask, patch_coords, region_prototypes):
    import sys
    if "/opt/trn_rl_repo" not in sys.path:
        sys.path.insert(0, "/opt/trn_rl_repo")
    from concourse.bass_utils import run_bass_kernel_spmd

    if "nc" not in _built:
        _built["nc"] = build()
    nc = _built["nc"]
    in_maps = make_in_maps(segmentation_mask, patch_coords)
    res = run_bass_kernel_spmd(nc, in_maps, list(range(NCORES))).results
    return assemble(res, region_prototypes)


# revision 18
# speedup vs baseline: 1.4894x; 1.1055x over previous
"""Trainium2 Bass kernel: AnatomicalStructureEncoder (histogram binning).

Pair-block gather variant:
  - 8 cores; core c handles batch b=c//4, patches [(c%4)*4096, +4096).
  - Patch p (local) = 2048*h + c', c' = 1024*gamma + c2.
  - SBUF partition q = 64h + 32*gamma + w2, w2 = dz*8 + ty: holds the
    528-voxel pair-block (rows dy=2ty and 2ty+1 of slice z0+dz) of patch
    (h, gamma, c2-column). One indirect DMA per c2 column moves 128
    blocks (one per partition) -> 1024 gather instructions total.
  - Window j (j=0,1) of a block sits at block offset 512*j..+16; a strided
    cast-copy compacts blocks to dm[128, c2, 2, 16] bf16, masks zero the
    x-tail and invalid windows, per-region is_equal one-hots are reduced
    by TensorE (block-ones stationary, 32 accumulating matmuls fold j,x).
  - Normalization: a_r = (counts_r + 1e-6*numel)/(sum counts + 2e-5*numel).
"""

import numpy as np

D, H, W = 64, 512, 512
HW = H * W
DHW = D * H * W
VPAD = 1024            # DRAM tail pad so 528-elem blocks never overrun
B = 2
NPTOT = 16384          # patches per batch
NCORES = 8
NP = 4096              # patches per core
NCH = NP // 2          # 2048 c' columns per half
C2 = NCH // 2          # 1024 c2 columns
CC = 128               # c2 columns per compute chunk
NCHUNK = C2 // CC      # 8
CG = 8                 # c2 columns per gather tile
BLK = 528              # pair-block elements (512 + 16)
R = 20
PW = 16
EMBED = 768

_built = {}


def _mk_ap(bass, ap_like, pattern, extra_off=0):
    return bass.AP(tensor=ap_like.tensor, offset=ap_like.offset + extra_off,
                   ap=pattern)


def build(debug=False):
    import contextlib

    import concourse.bass as bass
    import concourse.tile as tile
    from concourse import bacc, mybir

    f32 = mybir.dt.float32
    bf16 = mybir.dt.bfloat16
    i32 = mybir.dt.int32
    Alu = mybir.AluOpType

    nc = bacc.Bacc()
    i8 = mybir.dt.int8
    vol = nc.declare_dram_parameter("vol", [DHW + VPAD], i8, isOutput=False)
    coordsT = nc.declare_dram_parameter("coordsT", [3, 2, NCH], f32, isOutput=False)
    outp = nc.declare_dram_parameter("out", [2 * R, NCH], f32, isOutput=True)
    countsD = nc.dram_tensor("countsD", [2, 2, R, C2], mybir.dt.float32)

    with tile.TileContext(nc) as tc, contextlib.ExitStack() as ctx:
        consts = ctx.enter_context(tc.tile_pool(name="consts", bufs=1))
        keep = ctx.enter_context(tc.tile_pool(name="keep", bufs=1))
        mctx = contextlib.ExitStack()
        mathp = mctx.enter_context(tc.tile_pool(name="math", bufs=1))

        # ---- per-partition constants --------------------------------------
        # q = 64h + 32g + 8dz + ty
        qI = consts.tile([128, 1], i32)
        nc.gpsimd.iota(qI[:], [[1, 1]], channel_multiplier=1)
        dzI = consts.tile([128, 1], i32)
        nc.vector.tensor_scalar(out=dzI[:], in0=qI[:], scalar1=3, scalar2=3,
                                op0=Alu.logical_shift_right, op1=Alu.bitwise_and)
        dy0I = consts.tile([128, 1], i32)
        nc.vector.tensor_scalar(out=dy0I[:], in0=qI[:], scalar1=7, scalar2=1,
                                op0=Alu.bitwise_and, op1=Alu.logical_shift_left)
        dzF = consts.tile([128, 1], f32)
        nc.vector.tensor_copy(dzF[:], dzI[:])
        dy0F = consts.tile([128, 1], f32)
        nc.vector.tensor_copy(dy0F[:], dy0I[:])
        dy1F = consts.tile([128, 1], f32)
        nc.vector.tensor_scalar(out=dy1F[:], in0=dy0F[:], scalar1=1.0,
                                scalar2=None, op0=Alu.add)
        t01 = consts.tile([128, 1], f32)
        nc.vector.tensor_scalar(out=t01[:], in0=dy0F[:], scalar1=float(W),
                                scalar2=None, op0=Alu.mult)
        deltaF = consts.tile([128, 1], f32)
        nc.vector.scalar_tensor_tensor(out=deltaF[:], in0=dzF[:], scalar=float(HW),
                                       in1=t01[:], op0=Alu.mult, op1=Alu.add)

        iotaKI = consts.tile([128, PW], i32)
        nc.gpsimd.iota(iotaKI[:], [[1, PW]], channel_multiplier=0)
        iotaK = consts.tile([128, PW], bf16)
        nc.vector.tensor_copy(iotaK[:], iotaKI[:])

        # ones4 [128, 4] bf16: column m = (q >> 5) = 2h + gamma
        hgI = consts.tile([128, 1], i32)
        nc.vector.tensor_scalar(out=hgI[:], in0=qI[:], scalar1=5, scalar2=None,
                                op0=Alu.logical_shift_right)
        hgF = consts.tile([128, 1], f32)
        nc.vector.tensor_copy(hgF[:], hgI[:])
        ones4 = consts.tile([128, 4], bf16)
        for m in range(4):
            nc.vector.tensor_scalar(out=ones4[:, m:m + 1], in0=hgF[:],
                                    scalar1=float(m), scalar2=None,
                                    op0=Alu.is_equal)

        # counts partition layout: 20h + r.
        p40 = consts.tile([40, 1], i32)
        nc.gpsimd.iota(p40[:], [[1, 1]], channel_multiplier=1)
        p40f = consts.tile([40, 1], f32)
        nc.vector.tensor_copy(p40f[:], p40[:])
        h1c = consts.tile([40, 1], f32)
        nc.vector.tensor_scalar(out=h1c[:], in0=p40f[:], scalar1=19.5,
                                scalar2=None, op0=Alu.is_gt)
        lhsT40 = consts.tile([40, 2], f32)
        nc.vector.tensor_scalar(out=lhsT40[:, 0:1], in0=h1c[:], scalar1=-1.0,
                                scalar2=1.0, op0=Alu.mult, op1=Alu.add)
        nc.vector.tensor_copy(lhsT40[:, 1:2], h1c[:])

        i240 = consts.tile([2, 40], i32)
        nc.gpsimd.iota(i240[:], [[1, 40]], channel_multiplier=-20)
        i240f = consts.tile([2, 40], f32)
        nc.vector.tensor_copy(i240f[:], i240[:])
        i240a = consts.tile([2, 40], f32)
        nc.vector.tensor_scalar(out=i240a[:], in0=i240f[:], scalar1=-0.5,
                                scalar2=None, op0=Alu.is_gt)
        i240b2 = consts.tile([2, 40], f32)
        nc.vector.tensor_scalar(out=i240b2[:], in0=i240f[:], scalar1=19.5,
                                scalar2=None, op0=Alu.is_lt)
        lhsT240 = consts.tile([2, 40], f32)
        nc.vector.tensor_tensor(out=lhsT240[:], in0=i240a[:], in1=i240b2[:],
                                op=Alu.mult)

        # ---- coords, replicated: partition group (h, g) reads its c2 slice -
        cF = []
        for comp in range(3):
            t = mathp.tile([128, C2], f32, tag=f"c{comp}")
            for h in (0, 1):
                src = coordsT[comp, h, :]
                nc.gpsimd.dma_start(
                    out=t[64 * h:64 * h + 64, :],
                    in_=_mk_ap(bass, src, [[C2, 2], [0, 32], [1, C2]]),
                )
            cF.append(t)
        czF, cyF, cxF = cF

        # ---- coords math ---------------------------------------------------
        def floor_of(src, scale):
            pix = mathp.tile([128, C2], f32, tag="mt", bufs=8)
            nc.vector.tensor_scalar(out=pix[:], in0=src[:], scalar1=scale,
                                    scalar2=None, op0=Alu.mult)
            fi = mathp.tile([128, C2], i32, tag="mt", bufs=8)
            nc.vector.tensor_copy(fi[:], pix[:])
            ff = mathp.tile([128, C2], f32, tag="mt", bufs=8)
            nc.vector.tensor_copy(ff[:], fi[:])
            gt = mathp.tile([128, C2], f32, tag="mt", bufs=8)
            nc.vector.tensor_tensor(out=gt[:], in0=ff[:], in1=pix[:], op=Alu.is_gt)
            fl = mathp.tile([128, C2], f32, tag="mt", bufs=8)
            nc.vector.tensor_tensor(out=fl[:], in0=ff[:], in1=gt[:], op=Alu.subtract)
            return fl

        def axis(src, scale, half, dim, tagp):
            fl = floor_of(src, scale)
            s0 = mathp.tile([128, C2], f32, tag="s0" + tagp)
            nc.vector.tensor_scalar(out=s0[:], in0=fl[:], scalar1=-half,
                                    scalar2=0.0, op0=Alu.add, op1=Alu.max)
            se = mathp.tile([128, C2], f32, tag="mt", bufs=8)
            nc.vector.tensor_scalar(out=se[:], in0=fl[:], scalar1=half,
                                    scalar2=dim, op0=Alu.add, op1=Alu.min)
            L = mathp.tile([128, C2], f32, tag="L" + tagp)
            nc.vector.tensor_tensor(out=L[:], in0=se[:], in1=s0[:], op=Alu.subtract)
            return s0, L

        z0, Lz = axis(czF, float(D), 2.0, float(D), "z")
        y0, Ly = axis(cyF, float(H), 8.0, float(H), "y")
        x0, Lx = axis(cxF, float(W), 8.0, float(W), "x")

        numelF = mathp.tile([128, C2], f32)
        t1 = mathp.tile([128, C2], f32, tag="mt", bufs=8)
        nc.vector.tensor_tensor(out=t1[:], in0=Lz[:], in1=Ly[:], op=Alu.mult)
        nc.vector.tensor_tensor(out=numelF[:], in0=t1[:], in1=Lx[:], op=Alu.mult)

        b1 = mathp.tile([128, C2], f32, tag="mt", bufs=8)
        nc.vector.scalar_tensor_tensor(out=b1[:], in0=y0[:], scalar=float(W),
                                       in1=x0[:], op0=Alu.mult, op1=Alu.add)
        baseF = mathp.tile([128, C2], f32, tag="mt", bufs=8)
        nc.vector.scalar_tensor_tensor(out=baseF[:], in0=z0[:], scalar=float(HW),
                                       in1=b1[:], op0=Alu.mult, op1=Alu.add)
        offF = mathp.tile([128, C2], f32, tag="mt", bufs=8)
        nc.vector.tensor_tensor(out=offF[:], in0=baseF[:],
                                in1=deltaF[:, 0:1].to_broadcast([128, C2]),
                                op=Alu.add)

        vz = mathp.tile([128, C2], f32, tag="vz")
        nc.vector.tensor_tensor(out=vz[:], in0=dzF[:, 0:1].to_broadcast([128, C2]),
                                in1=Lz[:], op=Alu.is_ge)
        vy0 = mathp.tile([128, C2], f32, tag="vy0")
        nc.vector.tensor_tensor(out=vy0[:], in0=dy0F[:, 0:1].to_broadcast([128, C2]),
                                in1=Ly[:], op=Alu.is_ge)
        vy1 = mathp.tile([128, C2], f32, tag="vy1")
        nc.vector.tensor_tensor(out=vy1[:], in0=dy1F[:, 0:1].to_broadcast([128, C2]),
                                in1=Ly[:], op=Alu.is_ge)
        vinv0 = mathp.tile([128, C2], f32, tag="vinv0")
        nc.vector.tensor_tensor(out=vinv0[:], in0=vz[:], in1=vy0[:], op=Alu.max)
        vinv1 = mathp.tile([128, C2], f32, tag="vinv1")
        nc.vector.tensor_tensor(out=vinv1[:], in0=vz[:], in1=vy1[:], op=Alu.max)

        offF2 = mathp.tile([128, C2], f32, tag="mt", bufs=8)
        nc.vector.scalar_tensor_tensor(out=offF2[:], in0=vinv0[:], scalar=33554432.0,
                                       in1=offF[:], op0=Alu.mult, op1=Alu.add)
        offI = keep.tile([128, C2], i32)
        nc.vector.tensor_copy(offI[:], offF2[:])

        def lxe_of(vinv, tagn):
            vval = mathp.tile([128, C2], f32, tag="mt", bufs=8)
            nc.vector.tensor_scalar(out=vval[:], in0=vinv[:], scalar1=-1.0,
                                    scalar2=1.0, op0=Alu.mult, op1=Alu.add)
            lxe = mathp.tile([128, C2], f32, tag="mt", bufs=8)
            nc.vector.tensor_tensor(out=lxe[:], in0=Lx[:], in1=vval[:], op=Alu.mult)
            LxE = keep.tile([128, C2], bf16, tag=tagn)
            nc.vector.tensor_copy(LxE[:], lxe[:])
            return LxE

        LxE0 = lxe_of(vinv0, "lxe0")
        LxE1 = lxe_of(vinv1, "lxe1")

        # numel per (h, c') to 2 partitions: rows {0,32,64,96} -> c' halves
        NUM2 = keep.tile([2, NCH], f32)
        nc.sync.dma_start(out=NUM2[0:1, 0:C2], in_=numelF[0:1, :])
        nc.sync.dma_start(out=NUM2[0:1, C2:NCH], in_=numelF[32:33, :])
        nc.sync.dma_start(out=NUM2[1:2, 0:C2], in_=numelF[64:65, :])
        nc.sync.dma_start(out=NUM2[1:2, C2:NCH], in_=numelF[96:97, :])

        # ---- gather + histogram chunks ------------------------------------
        mctx.close()
        gpool = ctx.enter_context(tc.tile_pool(name="g", bufs=4))
        dpool = ctx.enter_context(tc.tile_pool(name="d", bufs=2))
        ohpool = ctx.enter_context(tc.tile_pool(name="oh", bufs=3))
        fpool = ctx.enter_context(tc.tile_pool(name="fold", bufs=2))
        pspool = ctx.enter_context(tc.tile_pool(name="ps", bufs=1, space="PSUM"))

        for j in range(NCHUNK):          # 8 chunks of CC=128 c2 columns
            c0 = j * CC
            dm = dpool.tile([128, CC, 2, PW], bf16, tag="dm")
            for s in range(CC // CG):    # 16 gather tiles of CG=8 columns
                gc0 = c0 + s * CG
                G2 = gpool.tile([128, CG, BLK], i8, tag="G2")
                g2b = G2[:]
                for u in range(CG):
                    out2d = bass.AP(tensor=g2b.tensor,
                                    offset=g2b.offset + u * BLK,
                                    ap=[g2b.ap[0], [1, BLK]])
                    nc.gpsimd.indirect_dma_start(
                        out=out2d,
                        out_offset=None,
                        in_=_mk_ap(bass, vol[:], [[1, DHW + VPAD], [1, 1]]),
                        in_offset=bass.IndirectOffsetOnAxis(
                            ap=offI[:, gc0 + u:gc0 + u + 1], axis=0),
                        bounds_check=DHW - 1,
                        oob_is_err=False,
                    )
                # compact-cast: dm[:, s*CG + v, j2, k] <- G2[:, v, 512*j2 + k]
                nc.vector.tensor_copy(
                    dm[:, s * CG:(s + 1) * CG, :, :],
                    _mk_ap(bass, G2[:], [g2b.ap[0], [BLK, CG], [512, 2], [1, PW]]),
                )
            m = dpool.tile([128, CC, 2, PW], bf16, tag="m", bufs=1)
            for j2, LxE in ((0, LxE0), (1, LxE1)):
                lxe_b = _mk_ap(bass, LxE[:, c0:c0 + CC],
                               [LxE[:, c0:c0 + CC].ap[0], [1, CC], [0, PW]])
                iota_b = _mk_ap(bass, iotaK[:, :],
                                [iotaK[:, :].ap[0], [0, CC], [1, PW]])
                nc.vector.tensor_tensor(out=m[:, :, j2, :], in0=lxe_b,
                                        in1=iota_b, op=Alu.is_gt)
            dm2 = dpool.tile([128, CC, 2, PW], bf16, tag="dm2")
            nc.vector.tensor_tensor(out=dm2[:], in0=dm[:], in1=m[:], op=Alu.mult)

            ps = pspool.tile([4, R, CC], f32, tag="ps")
            for r in range(1, R + 1):
                oh = ohpool.tile([128, CC, 2, PW], bf16, tag="oh")
                nc.vector.tensor_scalar(out=oh[:], in0=dm2[:], scalar1=float(r),
                                        scalar2=None, op0=Alu.is_equal)
                for j2 in range(2):
                    for k in range(PW):
                        nc.tensor.matmul(
                            out=ps[:, r - 1, :],
                            lhsT=ones4[:],
                            rhs=oh[:, :, j2:j2 + 1, k:k + 1],
                            start=(j2 == 0 and k == 0),
                            stop=(j2 == 1 and k == PW - 1),
                        )
            stage = fpool.tile([4, R, CC], f32, tag="stage")
            nc.scalar.activation(stage[:], ps[:],
                                 mybir.ActivationFunctionType.Copy)
            # stage[m=2h+g, r, c] -> countsD[h, g, r, c0+c]
            nc.sync.dma_start(
                out=_mk_ap(bass, countsD[:, :, :, :],
                           [[2 * R * C2, 2], [R * C2, 2], [C2, R], [1, CC]],
                           extra_off=c0),
                in_=stage[:],
            )

        # ---- normalization -------------------------------------------------
        counts = keep.tile([2 * R, NCH], f32)
        for g in range(2):
            for quarter in range(4):
                q0 = quarter * 256
                nc.sync.dma_start(
                    out=counts[:, g * C2 + q0:g * C2 + q0 + 256],
                    in_=_mk_ap(bass, countsD[:, :, :, :],
                               [[2 * R * C2, 2], [C2, R], [1, 256]],
                               extra_off=g * R * C2 + q0),
                )
        psS = pspool.tile([2, NCH], f32, tag="ps")
        for s4 in range(4):
            ssl = slice(s4 * 512, (s4 + 1) * 512)
            nc.tensor.matmul(out=psS[:, ssl], lhsT=lhsT40[:], rhs=counts[:, ssl],
                             start=True, stop=True)
        den2 = keep.tile([2, NCH], f32)
        nc.vector.scalar_tensor_tensor(out=den2[:], in0=NUM2[:], scalar=2.0e-5,
                                       in1=psS[:], op0=Alu.mult, op1=Alu.add)
        rcp2 = keep.tile([2, NCH], f32)
        nc.vector.reciprocal(rcp2[:], den2[:])

        psN = pspool.tile([2 * R, NCH], f32, tag="ps")
        for s4 in range(4):
            ssl = slice(s4 * 512, (s4 + 1) * 512)
            nc.tensor.matmul(out=psN[:, ssl], lhsT=lhsT240[:], rhs=NUM2[:, ssl],
                             start=True, stop=True)
        numer = keep.tile([2 * R, NCH], f32)
        for s4 in range(4):
            ssl = slice(s4 * 512, (s4 + 1) * 512)
            nc.vector.scalar_tensor_tensor(out=numer[:, ssl], in0=psN[:, ssl],
                                           scalar=1.0e-6, in1=counts[:, ssl],
                                           op0=Alu.mult, op1=Alu.add)

        psR = pspool.tile([2 * R, NCH], f32, tag="ps")
        for s4 in range(4):
            ssl = slice(s4 * 512, (s4 + 1) * 512)
            nc.tensor.matmul(out=psR[:, ssl], lhsT=lhsT240[:], rhs=rcp2[:, ssl],
                             start=True, stop=True)
        a40 = keep.tile([2 * R, NCH], f32)
        for s4 in range(4):
            ssl = slice(s4 * 512, (s4 + 1) * 512)
            nc.vector.tensor_tensor(out=a40[:, ssl], in0=numer[:, ssl],
                                    in1=psR[:, ssl], op=Alu.mult)
        nc.sync.dma_start(out=outp[:], in_=a40[:])

    nc.finalize()
    return nc


def make_in_maps(segmentation_mask, patch_coords):
    mask = np.asarray(segmentation_mask)
    coords = np.asarray(patch_coords)
    in_maps = []
    for core in range(NCORES):
        b = core // 4
        p0 = (core % 4) * NP
        volv = np.ascontiguousarray(mask[b, 0].reshape(-1)).astype(np.int8)
        volv = np.concatenate([volv, np.zeros(VPAD, np.int8)])
        csh = coords[b, p0:p0 + NP, :].astype(np.float32)        # [NP, 3]
        ct = np.ascontiguousarray(csh.T).reshape(3, 2, NCH).copy()
        in_maps.append({"vol": volv, "coordsT": ct})
    return in_maps


def assemble(results, region_prototypes):
    protos = np.asarray(region_prototypes).astype(np.float32)
    p2r = np.zeros((B, NPTOT, R), np.float32)
    for core in range(NCORES):
        o = np.asarray(results[core]["out"])                     # [2R, NCH]
        bidx = core // 4
        p0 = (core % 4) * NP
        arr = o.reshape(2, R, NCH).transpose(0, 2, 1).reshape(NP, R)
        p2r[bidx, p0:p0 + NP, :] = arr
    region_features = np.broadcast_to(protos[None], (B, R, EMBED)).copy()
    return region_features, p2r


def kernel(segmentation_mask, patch_coords, region_prototypes):
    import sys
    if "/opt/trn_rl_repo" not in sys.path:
        sys.path.insert(0, "/opt/trn_rl_repo")
    from concourse.bass_utils import run_bass_kernel_spmd

    if "nc" not in _built:
        _built["nc"] = build()
    nc = _built["nc"]
    in_maps = make_in_maps(segmentation_mask, patch_coords)
    res = run_bass_kernel_spmd(nc, in_maps, list(range(NCORES))).results
    return assemble(res, region_prototypes)


# revision 19
# speedup vs baseline: 1.7598x; 1.1816x over previous
"""Trainium2 Bass kernel: AnatomicalStructureEncoder (histogram binning).

Pair-block gather variant:
  - 8 cores; core c handles batch b=c//4, patches [(c%4)*4096, +4096).
  - Patch p (local) = 2048*h + c', c' = 1024*gamma + c2.
  - SBUF partition q = 64h + 32*gamma + w2, w2 = dz*8 + ty: holds the
    528-voxel pair-block (rows dy=2ty and 2ty+1 of slice z0+dz) of patch
    (h, gamma, c2-column). One indirect DMA per c2 column moves 128
    blocks (one per partition) -> 1024 gather instructions total.
  - Window j (j=0,1) of a block sits at block offset 512*j..+16; a strided
    cast-copy compacts blocks to dm[128, c2, 2, 16] bf16, masks zero the
    x-tail and invalid windows, per-region is_equal one-hots are reduced
    by TensorE (block-ones stationary, 32 accumulating matmuls fold j,x).
  - Normalization: a_r = (counts_r + 1e-6*numel)/(sum counts + 2e-5*numel).
"""

import numpy as np

D, H, W = 64, 512, 512
HW = H * W
DHW = D * H * W
VPAD = 1024            # DRAM tail pad so 528-elem blocks never overrun
B = 2
NPTOT = 16384          # patches per batch
NCORES = 8
NP = 4096              # patches per core
NCH = NP // 2          # 2048 c' columns per half
C2 = NCH // 2          # 1024 c2 columns
CC = 128               # c2 columns per compute chunk
NCHUNK = C2 // CC      # 8
CG = 8                 # c2 columns per gather tile
BLK = 528              # pair-block elements (512 + 16)
R = 20
PW = 16
EMBED = 768

_built = {}


def _mk_ap(bass, ap_like, pattern, extra_off=0):
    return bass.AP(tensor=ap_like.tensor, offset=ap_like.offset + extra_off,
                   ap=pattern)


def build(debug=False):
    import contextlib

    import concourse.bass as bass
    import concourse.tile as tile
    from concourse import bacc, mybir

    f32 = mybir.dt.float32
    bf16 = mybir.dt.bfloat16
    i32 = mybir.dt.int32
    Alu = mybir.AluOpType

    nc = bacc.Bacc()
    i8 = mybir.dt.int8
    vol = nc.declare_dram_parameter("vol", [DHW + VPAD], i8, isOutput=False)
    coordsT = nc.declare_dram_parameter("coordsT", [3, 2, NCH], f32, isOutput=False)
    outp = nc.declare_dram_parameter("out", [2 * R, NCH], f32, isOutput=True)
    countsD = nc.dram_tensor("countsD", [2, 2, R, C2], mybir.dt.float32)

    with tile.TileContext(nc) as tc, contextlib.ExitStack() as ctx:
        consts = ctx.enter_context(tc.tile_pool(name="consts", bufs=1))
        keep = ctx.enter_context(tc.tile_pool(name="keep", bufs=1))
        mctx = contextlib.ExitStack()
        mathp = mctx.enter_context(tc.tile_pool(name="math", bufs=1))

        # ---- per-partition constants --------------------------------------
        # q = 64h + 32g + 8dz + ty
        qI = consts.tile([128, 1], i32)
        nc.gpsimd.iota(qI[:], [[1, 1]], channel_multiplier=1)
        dzI = consts.tile([128, 1], i32)
        nc.vector.tensor_scalar(out=dzI[:], in0=qI[:], scalar1=3, scalar2=3,
                                op0=Alu.logical_shift_right, op1=Alu.bitwise_and)
        dy0I = consts.tile([128, 1], i32)
        nc.vector.tensor_scalar(out=dy0I[:], in0=qI[:], scalar1=7, scalar2=1,
                                op0=Alu.bitwise_and, op1=Alu.logical_shift_left)
        dzF = consts.tile([128, 1], f32)
        nc.vector.tensor_copy(dzF[:], dzI[:])
        dy0F = consts.tile([128, 1], f32)
        nc.vector.tensor_copy(dy0F[:], dy0I[:])
        dy1F = consts.tile([128, 1], f32)
        nc.vector.tensor_scalar(out=dy1F[:], in0=dy0F[:], scalar1=1.0,
                                scalar2=None, op0=Alu.add)
        t01 = consts.tile([128, 1], f32)
        nc.vector.tensor_scalar(out=t01[:], in0=dy0F[:], scalar1=float(W),
                                scalar2=None, op0=Alu.mult)
        deltaF = consts.tile([128, 1], f32)
        nc.vector.scalar_tensor_tensor(out=deltaF[:], in0=dzF[:], scalar=float(HW),
                                       in1=t01[:], op0=Alu.mult, op1=Alu.add)

        iotaKI = consts.tile([128, PW], i32)
        nc.gpsimd.iota(iotaKI[:], [[1, PW]], channel_multiplier=0)
        iotaK = consts.tile([128, PW], bf16)
        nc.vector.tensor_copy(iotaK[:], iotaKI[:])

        # ones4 [128, 4] bf16: column m = (q >> 5) = 2h + gamma
        hgI = consts.tile([128, 1], i32)
        nc.vector.tensor_scalar(out=hgI[:], in0=qI[:], scalar1=5, scalar2=None,
                                op0=Alu.logical_shift_right)
        hgF = consts.tile([128, 1], f32)
        nc.vector.tensor_copy(hgF[:], hgI[:])
        ones4 = consts.tile([128, 4], bf16)
        for m in range(4):
            nc.vector.tensor_scalar(out=ones4[:, m:m + 1], in0=hgF[:],
                                    scalar1=float(m), scalar2=None,
                                    op0=Alu.is_equal)

        # counts partition layout: 20h + r.
        p40 = consts.tile([40, 1], i32)
        nc.gpsimd.iota(p40[:], [[1, 1]], channel_multiplier=1)
        p40f = consts.tile([40, 1], f32)
        nc.vector.tensor_copy(p40f[:], p40[:])
        h1c = consts.tile([40, 1], f32)
        nc.vector.tensor_scalar(out=h1c[:], in0=p40f[:], scalar1=19.5,
                                scalar2=None, op0=Alu.is_gt)
        lhsT40 = consts.tile([40, 2], f32)
        nc.vector.tensor_scalar(out=lhsT40[:, 0:1], in0=h1c[:], scalar1=-1.0,
                                scalar2=1.0, op0=Alu.mult, op1=Alu.add)
        nc.vector.tensor_copy(lhsT40[:, 1:2], h1c[:])

        i240 = consts.tile([2, 40], i32)
        nc.gpsimd.iota(i240[:], [[1, 40]], channel_multiplier=-20)
        i240f = consts.tile([2, 40], f32)
        nc.vector.tensor_copy(i240f[:], i240[:])
        i240a = consts.tile([2, 40], f32)
        nc.vector.tensor_scalar(out=i240a[:], in0=i240f[:], scalar1=-0.5,
                                scalar2=None, op0=Alu.is_gt)
        i240b2 = consts.tile([2, 40], f32)
        nc.vector.tensor_scalar(out=i240b2[:], in0=i240f[:], scalar1=19.5,
                                scalar2=None, op0=Alu.is_lt)
        lhsT240 = consts.tile([2, 40], f32)
        nc.vector.tensor_tensor(out=lhsT240[:], in0=i240a[:], in1=i240b2[:],
                                op=Alu.mult)

        # ---- coords, replicated: partition group (h, g) reads its c2 slice -
        cF = []
        for comp in range(3):
            t = mathp.tile([128, C2], f32, tag=f"c{comp}")
            for h in (0, 1):
                src = coordsT[comp, h, :]
                nc.gpsimd.dma_start(
                    out=t[64 * h:64 * h + 64, :],
                    in_=_mk_ap(bass, src, [[C2, 2], [0, 32], [1, C2]]),
                )
            cF.append(t)
        czF, cyF, cxF = cF

        # ---- coords math ---------------------------------------------------
        def floor_of(src, scale):
            pix = mathp.tile([128, C2], f32, tag="mt", bufs=8)
            nc.vector.tensor_scalar(out=pix[:], in0=src[:], scalar1=scale,
                                    scalar2=None, op0=Alu.mult)
            fi = mathp.tile([128, C2], i32, tag="mt", bufs=8)
            nc.vector.tensor_copy(fi[:], pix[:])
            ff = mathp.tile([128, C2], f32, tag="mt", bufs=8)
            nc.vector.tensor_copy(ff[:], fi[:])
            gt = mathp.tile([128, C2], f32, tag="mt", bufs=8)
            nc.vector.tensor_tensor(out=gt[:], in0=ff[:], in1=pix[:], op=Alu.is_gt)
            fl = mathp.tile([128, C2], f32, tag="mt", bufs=8)
            nc.vector.tensor_tensor(out=fl[:], in0=ff[:], in1=gt[:], op=Alu.subtract)
            return fl

        def axis(src, scale, half, dim, tagp):
            fl = floor_of(src, scale)
            s0 = mathp.tile([128, C2], f32, tag="s0" + tagp)
            nc.vector.tensor_scalar(out=s0[:], in0=fl[:], scalar1=-half,
                                    scalar2=0.0, op0=Alu.add, op1=Alu.max)
            se = mathp.tile([128, C2], f32, tag="mt", bufs=8)
            nc.vector.tensor_scalar(out=se[:], in0=fl[:], scalar1=half,
                                    scalar2=dim, op0=Alu.add, op1=Alu.min)
            L = mathp.tile([128, C2], f32, tag="L" + tagp)
            nc.vector.tensor_tensor(out=L[:], in0=se[:], in1=s0[:], op=Alu.subtract)
            return s0, L

        z0, Lz = axis(czF, float(D), 2.0, float(D), "z")
        y0, Ly = axis(cyF, float(H), 8.0, float(H), "y")
        x0, Lx = axis(cxF, float(W), 8.0, float(W), "x")

        numelF = mathp.tile([128, C2], f32)
        t1 = mathp.tile([128, C2], f32, tag="mt", bufs=8)
        nc.vector.tensor_tensor(out=t1[:], in0=Lz[:], in1=Ly[:], op=Alu.mult)
        nc.vector.tensor_tensor(out=numelF[:], in0=t1[:], in1=Lx[:], op=Alu.mult)

        b1 = mathp.tile([128, C2], f32, tag="mt", bufs=8)
        nc.vector.scalar_tensor_tensor(out=b1[:], in0=y0[:], scalar=float(W),
                                       in1=x0[:], op0=Alu.mult, op1=Alu.add)
        baseF = mathp.tile([128, C2], f32, tag="mt", bufs=8)
        nc.vector.scalar_tensor_tensor(out=baseF[:], in0=z0[:], scalar=float(HW),
                                       in1=b1[:], op0=Alu.mult, op1=Alu.add)
        offF = mathp.tile([128, C2], f32, tag="mt", bufs=8)
        nc.vector.tensor_tensor(out=offF[:], in0=baseF[:],
                                in1=deltaF[:, 0:1].to_broadcast([128, C2]),
                                op=Alu.add)

        vz = mathp.tile([128, C2], f32, tag="vz")
        nc.vector.tensor_tensor(out=vz[:], in0=dzF[:, 0:1].to_broadcast([128, C2]),
                                in1=Lz[:], op=Alu.is_ge)
        vy0 = mathp.tile([128, C2], f32, tag="vy0")
        nc.vector.tensor_tensor(out=vy0[:], in0=dy0F[:, 0:1].to_broadcast([128, C2]),
                                in1=Ly[:], op=Alu.is_ge)
        vy1 = mathp.tile([128, C2], f32, tag="vy1")
        nc.vector.tensor_tensor(out=vy1[:], in0=dy1F[:, 0:1].to_broadcast([128, C2]),
                                in1=Ly[:], op=Alu.is_ge)
        vinv0 = mathp.tile([128, C2], f32, tag="vinv0")
        nc.vector.tensor_tensor(out=vinv0[:], in0=vz[:], in1=vy0[:], op=Alu.max)
        vinv1 = mathp.tile([128, C2], f32, tag="vinv1")
        nc.vector.tensor_tensor(out=vinv1[:], in0=vz[:], in1=vy1[:], op=Alu.max)

        offF2 = mathp.tile([128, C2], f32, tag="mt", bufs=8)
        nc.vector.scalar_tensor_tensor(out=offF2[:], in0=vinv0[:], scalar=33554432.0,
                                       in1=offF[:], op0=Alu.mult, op1=Alu.add)
        offI = keep.tile([128, C2], i32)
        nc.vector.tensor_copy(offI[:], offF2[:])

        def lxe_of(vinv, tagn):
            vval = mathp.tile([128, C2], f32, tag="mt", bufs=8)
            nc.vector.tensor_scalar(out=vval[:], in0=vinv[:], scalar1=-1.0,
                                    scalar2=1.0, op0=Alu.mult, op1=Alu.add)
            lxe = mathp.tile([128, C2], f32, tag="mt", bufs=8)
            nc.vector.tensor_tensor(out=lxe[:], in0=Lx[:], in1=vval[:], op=Alu.mult)
            LxE = keep.tile([128, C2], bf16, tag=tagn)
            nc.vector.tensor_copy(LxE[:], lxe[:])
            return LxE

        LxE0 = lxe_of(vinv0, "lxe0")
        LxE1 = lxe_of(vinv1, "lxe1")

        # numel per (h, c') to 2 partitions: rows {0,32,64,96} -> c' halves
        NUM2 = keep.tile([2, NCH], f32)
        nc.sync.dma_start(out=NUM2[0:1, 0:C2], in_=numelF[0:1, :])
        nc.sync.dma_start(out=NUM2[0:1, C2:NCH], in_=numelF[32:33, :])
        nc.sync.dma_start(out=NUM2[1:2, 0:C2], in_=numelF[64:65, :])
        nc.sync.dma_start(out=NUM2[1:2, C2:NCH], in_=numelF[96:97, :])

        # ---- gather + histogram chunks ------------------------------------
        mctx.close()
        gpool = ctx.enter_context(tc.tile_pool(name="g", bufs=6))
        dpool = ctx.enter_context(tc.tile_pool(name="d", bufs=2))
        ohpool = ctx.enter_context(tc.tile_pool(name="oh", bufs=3))
        fpool = ctx.enter_context(tc.tile_pool(name="fold", bufs=2))
        pspool = ctx.enter_context(tc.tile_pool(name="ps", bufs=1, space="PSUM"))

        for j in range(NCHUNK):          # 8 chunks of CC=128 c2 columns
            c0 = j * CC
            dm = dpool.tile([128, CC, 2, PW], bf16, tag="dm")
            for s in range(CC // CG):    # 16 gather tiles of CG=8 columns
                gc0 = c0 + s * CG
                G2 = gpool.tile([128, CG, BLK], i8, tag="G2")
                g2b = G2[:]
                for u in range(CG):
                    out2d = bass.AP(tensor=g2b.tensor,
                                    offset=g2b.offset + u * BLK,
                                    ap=[g2b.ap[0], [1, BLK]])
                    nc.gpsimd.indirect_dma_start(
                        out=out2d,
                        out_offset=None,
                        in_=_mk_ap(bass, vol[:], [[1, DHW + VPAD], [1, 1]]),
                        in_offset=bass.IndirectOffsetOnAxis(
                            ap=offI[:, gc0 + u:gc0 + u + 1], axis=0),
                        bounds_check=DHW - 1,
                        oob_is_err=False,
                    )
                # compact-cast on ScalarE (keeps G2 recycling off the
                # busy Vector queue): dm[...] <- G2[:, v, 512*j2 + k]
                nc.scalar.activation(
                    dm[:, s * CG:(s + 1) * CG, :, :],
                    _mk_ap(bass, G2[:], [g2b.ap[0], [BLK, CG], [512, 2], [1, PW]]),
                    mybir.ActivationFunctionType.Copy,
                )
            m = dpool.tile([128, CC, 2, PW], bf16, tag="m", bufs=1)
            for j2, LxE in ((0, LxE0), (1, LxE1)):
                lxe_b = _mk_ap(bass, LxE[:, c0:c0 + CC],
                               [LxE[:, c0:c0 + CC].ap[0], [1, CC], [0, PW]])
                iota_b = _mk_ap(bass, iotaK[:, :],
                                [iotaK[:, :].ap[0], [0, CC], [1, PW]])
                nc.vector.tensor_tensor(out=m[:, :, j2, :], in0=lxe_b,
                                        in1=iota_b, op=Alu.is_gt)
            dm2 = dpool.tile([128, CC, 2, PW], bf16, tag="dm2")
            nc.vector.tensor_tensor(out=dm2[:], in0=dm[:], in1=m[:], op=Alu.mult)

            ps = pspool.tile([4, R, CC], f32, tag="ps")
            for r in range(1, R + 1):
                oh = ohpool.tile([128, CC, 2, PW], bf16, tag="oh")
                nc.vector.tensor_scalar(out=oh[:], in0=dm2[:], scalar1=float(r),
                                        scalar2=None, op0=Alu.is_equal)
                for j2 in range(2):
                    for k in range(PW):
                        nc.tensor.matmul(
                            out=ps[:, r - 1, :],
                            lhsT=ones4[:],
                            rhs=oh[:, :, j2:j2 + 1, k:k + 1],
                            start=(j2 == 0 and k == 0),
                            stop=(j2 == 1 and k == PW - 1),
                        )
            stage = fpool.tile([4, R, CC], f32, tag="stage")
            nc.scalar.activation(stage[:], ps[:],
                                 mybir.ActivationFunctionType.Copy)
            # stage[m=2h+g, r, c] -> countsD[h, g, r, c0+c]
            nc.sync.dma_start(
                out=_mk_ap(bass, countsD[:, :, :, :],
                           [[2 * R * C2, 2], [R * C2, 2], [C2, R], [1, CC]],
                           extra_off=c0),
                in_=stage[:],
            )

        # ---- normalization -------------------------------------------------
        counts = keep.tile([2 * R, NCH], f32)
        for g in range(2):
            for quarter in range(4):
                q0 = quarter * 256
                nc.sync.dma_start(
                    out=counts[:, g * C2 + q0:g * C2 + q0 + 256],
                    in_=_mk_ap(bass, countsD[:, :, :, :],
                               [[2 * R * C2, 2], [C2, R], [1, 256]],
                               extra_off=g * R * C2 + q0),
                )
        psS = pspool.tile([2, NCH], f32, tag="ps")
        for s4 in range(4):
            ssl = slice(s4 * 512, (s4 + 1) * 512)
            nc.tensor.matmul(out=psS[:, ssl], lhsT=lhsT40[:], rhs=counts[:, ssl],
                             start=True, stop=True)
        den2 = keep.tile([2, NCH], f32)
        nc.vector.scalar_tensor_tensor(out=den2[:], in0=NUM2[:], scalar=2.0e-5,
                                       in1=psS[:], op0=Alu.mult, op1=Alu.add)
        rcp2 = keep.tile([2, NCH], f32)
        nc.vector.reciprocal(rcp2[:], den2[:])

        psN = pspool.tile([2 * R, NCH], f32, tag="ps")
        for s4 in range(4):
            ssl = slice(s4 * 512, (s4 + 1) * 512)
            nc.tensor.matmul(out=psN[:, ssl], lhsT=lhsT240[:], rhs=NUM2[:, ssl],
                             start=True, stop=True)
        numer = keep.tile([2 * R, NCH], f32)
        for s4 in range(4):
            ssl = slice(s4 * 512, (s4 + 1) * 512)
            nc.vector.scalar_tensor_tensor(out=numer[:, ssl], in0=psN[:, ssl],
                                           scalar=1.0e-6, in1=counts[:, ssl],
                                           op0=Alu.mult, op1=Alu.add)

        psR = pspool.tile([2 * R, NCH], f32, tag="ps")
        for s4 in range(4):
            ssl = slice(s4 * 512, (s4 + 1) * 512)
            nc.tensor.matmul(out=psR[:, ssl], lhsT=lhsT240[:], rhs=rcp2[:, ssl],
                             start=True, stop=True)
        a40 = keep.tile([2 * R, NCH], f32)
        for s4 in range(4):
            ssl = slice(s4 * 512, (s4 + 1) * 512)
            nc.vector.tensor_tensor(out=a40[:, ssl], in0=numer[:, ssl],
                                    in1=psR[:, ssl], op=Alu.mult)
        nc.sync.dma_start(out=outp[:], in_=a40[:])

    nc.finalize()
    return nc


def make_in_maps(segmentation_mask, patch_coords):
    mask = np.asarray(segmentation_mask)
    coords = np.asarray(patch_coords)
    in_maps = []
    for core in range(NCORES):
        b = core // 4
        p0 = (core % 4) * NP
        volv = np.ascontiguousarray(mask[b, 0].reshape(-1)).astype(np.int8)
        volv = np.concatenate([volv, np.zeros(VPAD, np.int8)])
        csh = coords[b, p0:p0 + NP, :].astype(np.float32)        # [NP, 3]
        ct = np.ascontiguousarray(csh.T).reshape(3, 2, NCH).copy()
        in_maps.append({"vol": volv, "coordsT": ct})
    return in_maps


def assemble(results, region_prototypes):
    protos = np.asarray(region_prototypes).astype(np.float32)
    p2r = np.zeros((B, NPTOT, R), np.float32)
    for core in range(NCORES):
        o = np.asarray(results[core]["out"])                     # [2R, NCH]
        bidx = core // 4
        p0 = (core % 4) * NP
        arr = o.reshape(2, R, NCH).transpose(0, 2, 1).reshape(NP, R)
        p2r[bidx, p0:p0 + NP, :] = arr
    region_features = np.broadcast_to(protos[None], (B, R, EMBED)).copy()
    return region_features, p2r


def kernel(segmentation_mask, patch_coords, region_prototypes):
    import sys
    if "/opt/trn_rl_repo" not in sys.path:
        sys.path.insert(0, "/opt/trn_rl_repo")
    from concourse.bass_utils import run_bass_kernel_spmd

    if "nc" not in _built:
        _built["nc"] = build()
    nc = _built["nc"]
    in_maps = make_in_maps(segmentation_mask, patch_coords)
    res = run_bass_kernel_spmd(nc, in_maps, list(range(NCORES))).results
    return assemble(res, region_prototypes)
